# revision 48
# baseline (speedup 1.0000x reference)
"""HolE scorer kernel for 8 Trainium2 NeuronCores (Bass/Tile).

Computation (reference):
    a = x @ W_e.T; b = y @ W_e.T; rr = r @ W_r.T          # (B, d)
    corr = irfft(rfft(a) * conj(rfft(b))) / d             # circular correlation
    out = sigmoid(sum(rr * corr, axis=1))                 # (B, 1)

Strategy:
  - Tensor-parallel over entities for the two big GEMMs: core c holds
    entity columns [c*12500, (c+1)*12500) of x, y, W_e (padded to 12544 =
    98*128), computing partial a.T/b.T (d-major) in fp8 e4m3 with
    DoubleRow perf mode (two 128-K tiles per instruction at 2x bf16 rate).
    W_e is pre-scaled by 2^14 into fp8 normal range; the inverse scale is
    folded into the a/b DFT basis (exact power-of-2).  Validated max rel
    err ~1.6e-2 on the final sigmoid output (tolerance 2e-2).
  - Batch columns are processed in 4 passes: y[0:512], y[512:1024],
    x[0:512], x[512:1024].  Each pass ends in a ReduceScatter(add)
    handing each core its 64 columns of fully-summed output; every RS
    except the final one hides under the next pass's GEMM.
  - Core c owns batch rows {c*64..+63, 512+c*64..+63} (local rows
    0:64, 64:128) — host gathers accordingly.
  - Tail per core (128 batch rows): rr.T GEMM, rfft via DFT-basis matmuls,
    and the irfft+rowwise-dot folded into a frequency-domain weighted dot
    (Parseval):  score_i = (1/d^2) sum_f w_f (Rr*Pr + Ri*Pi)[i,f],
    with P = A * conj(B), w = [1, 2, ..., 2, 1].  The w/d^2 factor is
    folded into the rr DFT basis; the rfft/score of each a-chunk runs as
    soon as that chunk's RS lands, overlapping the remaining GEMM passes.
  - All DRAM operands are staged host-side in partition-major layout
    (128, k-chunks, cols) so every DMA descriptor is a contiguous
    multi-KB run per partition.
  - Queue split: W_e/static/staging/tail DMAs ride the Scalar HWDGE
    queue, the streamed x/y tiles the Sync queue.
"""

import numpy as np
import ml_dtypes

import concourse.bass as bass
import concourse.tile as tile
from concourse import bacc, mybir
from concourse.alu_op_type import AluOpType
from concourse.bass_utils import run_bass_kernel_spmd

# Problem shapes (hardcoded per contract)
B = 1024            # batch
D = 512             # num_dim
E = 100000          # num_entities
R = 1000            # num_relations
NCORES = 8

E_SH = E // NCORES          # 12500 entities per core
KC = 98                     # k-chunks of 128 after padding (98*128 = 12544)
E_PAD = KC * 128            # 12544
KG = 7                      # k-groups
KJ = KC // KG               # 14 chunks per group
RC = 8                      # relation k-chunks (1000 -> 1024)
R_PAD = RC * 128
NF = D // 2 + 1             # 257 rfft bins
B_SH = B // NCORES          # 128 batch rows per core

# batch-column chunks: (col0, ncols); per-core share w = ncols // 8.
CHUNKS = [(0, 512), (512, 512)]
ROW_OFF = [0, 64]

BF16 = mybir.dt.bfloat16
F32 = mybir.dt.float32
F8 = mybir.dt.float8e4
S_WE = 16384.0              # 2^14: lifts W_e (~3e-3) into fp8 normal range

_cached = {}


def _dft_bases():
    d = D
    dd = np.arange(d, dtype=np.float64)[:, None]
    ff = np.arange(NF, dtype=np.float64)[None, :]
    ang = 2.0 * np.pi * dd * ff / d
    fr = np.cos(ang)
    fi = -np.sin(ang)
    # a/b arrive scaled by S_WE (fp8 W_e pre-scale); fold 1/S_WE here.
    f_ab = np.concatenate([fr, fi], axis=1) / S_WE       # (512, 514)
    w = np.full(NF, 2.0); w[0] = 1.0; w[-1] = 1.0
    scale = w / (d * d)
    f_r = np.concatenate([fr * scale, fi * scale], axis=1)
    return (f_ab.astype(ml_dtypes.bfloat16), f_r.astype(ml_dtypes.bfloat16))


def _build_program():
    nc = bacc.Bacc("TRN2", target_bir_lowering=False, debug=False,
                   num_devices=NCORES)

    # partition-major DRAM operands: (128, k-chunks, cols)
    x_d = [nc.dram_tensor(f"x{ci}", (128, KC, nc_), F8, kind="ExternalInput")
           for ci, (_, nc_) in enumerate(CHUNKS)]
    y_d = [nc.dram_tensor(f"y{ci}", (128, KC, nc_), F8, kind="ExternalInput")
           for ci, (_, nc_) in enumerate(CHUNKS)]
    weT_d = nc.dram_tensor("weT", (128, KC, D), F8, kind="ExternalInput")
    rT_d = nc.dram_tensor("rT", (128, RC, B_SH), BF16, kind="ExternalInput")
    wrT_d = nc.dram_tensor("wrT", (128, RC, D), BF16, kind="ExternalInput")
    fab_d = nc.dram_tensor("fab", (128, 4, 2 * NF), BF16, kind="ExternalInput")
    fr_d = nc.dram_tensor("fr", (128, 4, 2 * NF), BF16, kind="ExternalInput")
    out_d = nc.dram_tensor("out", (B_SH, 1), F32, kind="ExternalOutput")

    # staging + reduce-scatter outputs.  Both b passes share one mesh
    # (key "b01", their spectra are needed only after the a0 pass); the
    # last pass is reduce-scattered in two K-stages ("a1A"/"a1B").
    stages = {}
    rs_outs = {}
    for key, w_ in (("b01", 128), ("a0", 64), ("a1A", 64), ("a1B", 64)):
        stages[key] = nc.dram_tensor(f"stage_{key}", (NCORES, D, w_), BF16)
        rs_outs[key] = nc.dram_tensor(f"rs_{key}", (D, w_), BF16)
    groups = [list(range(NCORES))]

    with tile.TileContext(nc) as tc:
        with (
            tc.tile_pool(name="weights", bufs=1) as wpool,
            tc.tile_pool(name="stream", bufs=7) as spool,
            tc.tile_pool(name="copies", bufs=4) as cpool,
            tc.tile_pool(name="tail", bufs=1) as tpool,
            tc.tile_pool(name="psum", bufs=8, space="PSUM") as ppool,
        ):
            # ---- resident W_e.T groups (Scalar queue, we0 gated fine) ----
            we_tiles = []
            for g in range(KG):
                wt = wpool.tile([128, KJ, D], F8, tag=f"we{g}", name=f"we{g}")
                src = weT_d[:, g * KJ:(g + 1) * KJ, :]
                if g == 0:
                    nc.scalar.dma_start(wt[:, 0:2], src[:, 0:2])
                    nc.scalar.dma_start(wt[:, 2:4], src[:, 2:4])
                    nc.scalar.dma_start(wt[:, 4:8], src[:, 4:8])
                    nc.scalar.dma_start(wt[:, 8:KJ], src[:, 8:KJ])
                else:
                    nc.scalar.dma_start(wt[:], src)
                we_tiles.append(wt)

            # small static tensors, needed only mid-kernel (Scalar queue)
            r_t = wpool.tile([128, RC, B_SH], BF16, tag="r", name="r")
            nc.scalar.dma_start(r_t[:], rT_d[:])
            wr_t = wpool.tile([128, RC, D], BF16, tag="wr", name="wr")
            nc.scalar.dma_start(wr_t[:], wrT_d[:])
            fab_t = wpool.tile([128, 4, 2 * NF], BF16, tag="fab", name="fab")
            nc.scalar.dma_start(fab_t[:], fab_d[:])
            fr_t = wpool.tile([128, 4, 2 * NF], BF16, tag="frq", name="frq")
            nc.scalar.dma_start(fr_t[:], fr_d[:])

            rr_b = tpool.tile([128, 4, B_SH], BF16, name="rr_b")

            def load_chunk(key, w_=64):
                tb = tpool.tile([128, 4, w_], BF16, name=f"ld_{key}")
                nc.scalar.dma_start(
                    tb[:],
                    rs_outs[key][:].rearrange("(mc p) q -> p mc q", p=128))
                return tb

            def rfft_mm(src_b, basis, psr, psi, lo, w_,
                        start=True, stop=True):
                for k in range(4):
                    nc.tensor.matmul(psr[lo:lo + w_, :], src_b[:, k, :],
                                     basis[:, k, 0:NF],
                                     start=(start and k == 0),
                                     stop=(stop and k == 3))
                for k in range(4):
                    nc.tensor.matmul(psi[lo:lo + w_, :], src_b[:, k, :],
                                     basis[:, k, NF:2 * NF],
                                     start=(start and k == 0),
                                     stop=(stop and k == 3))

            f1 = tpool.tile([B_SH, NF], F32, name="f1")
            f2 = tpool.tile([B_SH, NF], F32, name="f2")
            g_t = tpool.tile([B_SH, 2 * NF], F32, name="g_t")
            sig = tpool.tile([B_SH, 1], F32, name="sig")
            ps_ar = None
            ps_ai = None
            a_tiles = {}

            def a_score_tail(ci, tiles):
                """rfft chunk ci of a (from partial-sum tiles, summed via
                PSUM accumulation), combine with F1/F2, score+sigmoid."""
                lo = ROW_OFF[ci]
                for ti, a_tile in enumerate(tiles):
                    rfft_mm(a_tile, fab_t, ps_ar, ps_ai, lo, 64,
                            start=(ti == 0), stop=(ti == len(tiles) - 1))
                sl = slice(lo, lo + 64)
                nc.vector.tensor_tensor(g_t[sl, 0:NF], ps_ar[sl], f1[sl],
                                        AluOpType.mult)
                nc.vector.tensor_tensor(g_t[sl, NF:2 * NF], ps_ai[sl],
                                        f2[sl], AluOpType.mult)
                score = tpool.tile([64, 1], F32, tag="score",
                                   name=f"score{ci}")
                nc.vector.reduce_sum(score[:], g_t[sl, :],
                                     axis=mybir.AxisListType.X)
                nc.scalar.activation(sig[sl], score[:],
                                     mybir.ActivationFunctionType.Sigmoid)
                nc.sync.dma_start(out_d[sl, :], sig[sl])

            def emit_gemm(mat_d, accs, g_lo, g_hi, kp_lo, kp_hi, nm,
                          gate_first=False):
                for g in range(g_lo, g_hi):
                    xt = spool.tile([128, KJ, 512], F8, tag="xs",
                                    name=f"xs{nm}{g}")
                    src = mat_d[:, g * KJ:(g + 1) * KJ, :]
                    if gate_first and g == 0:
                        nc.sync.dma_start(xt[:, 0:2], src[:, 0:2])
                        nc.sync.dma_start(xt[:, 2:4], src[:, 2:4])
                        nc.sync.dma_start(xt[:, 4:8], src[:, 4:8])
                        nc.sync.dma_start(xt[:, 8:KJ], src[:, 8:KJ])
                    else:
                        nc.sync.dma_start(xt[:], src)
                    for j2 in range(0, KJ, 2):
                        kp = (g * KJ + j2) // 2
                        for m in range(4):
                            nc.tensor.matmul(
                                accs[m][:],
                                we_tiles[g][:, j2:j2 + 2,
                                            m * 128:(m + 1) * 128],
                                xt[:, j2:j2 + 2, :],
                                start=(kp == kp_lo), stop=(kp == kp_hi),
                                perf_mode=mybir.MatmulPerfMode.DoubleRow)

            def emit_stage(accs, key, nm, q0=0, queues=None, order=None):
                # queues: per-m DMA-issuing engines; the partition->core
                # scatter emits many small descriptors, so spreading the
                # four m-tiles over multiple queues cuts its latency.
                # order: copy emission order — put the tiles bound for the
                # least-busy queue first so their DMAs start earliest.
                queues = queues or [nc.scalar] * 4
                for m in order or range(4):
                    sb = cpool.tile([128, 512], BF16, tag="cp",
                                    name=f"cp{nm}{m}")
                    nc.vector.tensor_copy(sb[:], accs[m][:])
                    dst = (stages[key][:, m * 128:(m + 1) * 128,
                                       q0:q0 + 64]
                           .rearrange("c d q -> d c q"))
                    src = sb.rearrange("d (c q) -> d c q", c=NCORES)
                    queues[m].dma_start(dst, src)

            def emit_rs(key):
                nc.gpsimd.collective_compute(
                    "ReduceScatter", AluOpType.add,
                    replica_groups=groups,
                    ins=[stages[key][:].opt()],
                    outs=[rs_outs[key][:].opt()])

            # ---- main GEMM passes ----
            passes = [("b", 0), ("b", 1), ("a", 0)]
            for pi_, (mat, ci) in enumerate(passes):
                mat_d = y_d[ci] if mat == "b" else x_d[ci]
                key = f"{mat}{ci}"
                accs = [
                    ppool.tile([128, 512], F32, tag="acc",
                               name=f"acc{key}{m}")
                    for m in range(4)
                ]
                emit_gemm(mat_d, accs, 0, KG, 0, KC // 2 - 1, key,
                          gate_first=(pi_ == 0))
                if mat == "b":
                    emit_stage(accs, "b01", key, q0=64 * ci)
                    if ci == 1:
                        emit_rs("b01")
                else:
                    emit_stage(accs, key, key)
                    emit_rs(key)

                if pi_ == 0:
                    # rr.T GEMM slotted after the first pass: its inputs are
                    # small and arrive behind that pass's stream DMAs.
                    ps_rr = ppool.tile([128, 4, B_SH], F32, tag="acc",
                                       name="ps_rr")
                    for m in range(4):
                        for j in range(RC):
                            nc.tensor.matmul(
                                ps_rr[:, m, :],
                                wr_t[:, j, m * 128:(m + 1) * 128],
                                r_t[:, j, :],
                                start=(j == 0), stop=(j == RC - 1))
                    nc.vector.tensor_copy(rr_b[:], ps_rr[:])

                if pi_ == 2:
                    # prefetch the merged b rs-output now: it sits on the
                    # scalar FIFO after this pass's staging, so it fires as
                    # soon as the b01 mesh ends without blocking anything.
                    b01_tile = load_chunk("b01", w_=128)

            # ---- last pass a[512:1024], reduce-scattered in two K-stages:
            # stage A (k-pairs 0..20) triggers its RS mid-pass so the mesh
            # latency and inter-core skew are absorbed under stage B's GEMM
            # (k-pairs 21..48); the final RS then starts with cores already
            # aligned and only ~12us of mesh is exposed.  The two partial
            # sums recombine for free inside the rfft's PSUM accumulation.
            KPA = 3 * KJ // 2 - 1                 # last k-pair of stage A
            # allocate both stages' PSUM up front: accsB lands on banks
            # freed at the a0 pass's end, so stage B's GEMM never waits on
            # stage A's copies.
            accsA = [ppool.tile([128, 512], F32, tag="acc",
                                name=f"accA{m}") for m in range(4)]
            accsB = [ppool.tile([128, 512], F32, tag="acc",
                                name=f"accB{m}") for m in range(4)]
            emit_gemm(x_d[1], accsA, 0, 3, 0, KPA, "a1A")
            # stage B's stream tiles: issued now (behind stage A's on the
            # sync queue), kept resident — stage B's GEMM runs m-major and
            # re-reads each tile four times.
            xtB = []
            for g in range(3, KG):
                xt = spool.tile([128, KJ, 512], F8, tag="xs",
                                name=f"xsa1B{g}")
                nc.sync.dma_start(xt[:], x_d[1][:, g * KJ:(g + 1) * KJ, :])
                xtB.append(xt)
            emit_stage(accsA, "a1A", "a1A")
            emit_rs("a1A")

            # stage B GEMM, m-major: each m-tile's accumulation finishes a
            # quarter of the way through, so its copy + staging scatter
            # overlap the remaining GEMM — only m3's staging (~5us) is
            # left between the last matmul and the final mesh trigger.
            bq = [nc.scalar, nc.sync, nc.scalar, nc.sync]
            for m in range(4):
                for g in range(3, KG):
                    for j2 in range(0, KJ, 2):
                        kp = (g * KJ + j2) // 2
                        nc.tensor.matmul(
                            accsB[m][:],
                            we_tiles[g][:, j2:j2 + 2,
                                        m * 128:(m + 1) * 128],
                            xtB[g - 3][:, j2:j2 + 2, :],
                            start=(kp == KPA + 1), stop=(kp == KC // 2 - 1),
                            perf_mode=mybir.MatmulPerfMode.DoubleRow)
                sb = cpool.tile([128, 512], BF16, tag="cp", name=f"cpB{m}")
                nc.vector.tensor_copy(sb[:], accsB[m][:])
                dst = (stages["a1B"][:, m * 128:(m + 1) * 128, :]
                       .rearrange("c d q -> d c q"))
                bq[m].dma_start(dst, sb.rearrange("d (c q) -> d c q",
                                                  c=NCORES))
            emit_rs("a1B")
            a_tiles[0] = load_chunk("a0")
            a_tiles[1] = load_chunk("a1A")
            a_tiles[2] = load_chunk("a1B")

            # ---- tail: all spectrum work overlaps the exposed final
            # mesh.  By now every earlier mesh has had 90us+ to complete,
            # so even pathological mesh spans cannot stall the GEMMs.
            # F1 = Rr.Br - Ri.Bi, F2 = Rr.Bi + Ri.Br.
            ps_br = ppool.tile([B_SH, NF], F32, tag="acc", name="ps_br")
            ps_bi = ppool.tile([B_SH, NF], F32, tag="acc", name="ps_bi")
            ps_qr = ppool.tile([B_SH, NF], F32, tag="acc", name="ps_qr")
            ps_qi = ppool.tile([B_SH, NF], F32, tag="acc", name="ps_qi")
            for bci in range(2):
                rfft_mm(b01_tile[:, :, 64 * bci:64 * (bci + 1)],
                        fab_t, ps_br, ps_bi, ROW_OFF[bci], 64)
            rfft_mm(rr_b, fr_t, ps_qr, ps_qi, 0, B_SH)
            s_qr = tpool.tile([B_SH, NF], F32, name="s_qr")
            nc.vector.tensor_copy(s_qr[:], ps_qr[:])
            s_qi = tpool.tile([B_SH, NF], F32, name="s_qi")
            nc.vector.tensor_copy(s_qi[:], ps_qi[:])
            t1 = tpool.tile([B_SH, NF], F32, name="t1")
            t2 = tpool.tile([B_SH, NF], F32, name="t2")
            nc.vector.tensor_tensor(f1[:], ps_br[:], s_qr[:],
                                    AluOpType.mult)
            nc.vector.tensor_tensor(t1[:], ps_bi[:], s_qi[:],
                                    AluOpType.mult)
            nc.vector.tensor_tensor(f1[:], f1[:], t1[:],
                                    AluOpType.subtract)
            nc.vector.tensor_tensor(f2[:], ps_bi[:], s_qr[:],
                                    AluOpType.mult)
            nc.vector.tensor_tensor(t2[:], ps_br[:], s_qi[:],
                                    AluOpType.mult)
            nc.vector.tensor_tensor(f2[:], f2[:], t2[:],
                                    AluOpType.add)

            ps_ar = ppool.tile([B_SH, NF], F32, tag="acc", name="ps_ar")
            ps_ai = ppool.tile([B_SH, NF], F32, tag="acc", name="ps_ai")
            a_score_tail(0, [a_tiles[0]])
            a_score_tail(1, [a_tiles[1], a_tiles[2]])

    nc.compile()
    return nc


def _get_program():
    if "nc" not in _cached:
        _cached["nc"] = _build_program()
    return _cached["nc"]


def _core_rows(c):
    """Batch rows owned by core c (matches CHUNKS/ROW_OFF mapping)."""
    return np.r_[c * 64:(c + 1) * 64,
                 512 + c * 64:512 + (c + 1) * 64]


def _pmajor(arr, kc):
    """(kc*128, cols) -> partition-major (128, kc, cols), contiguous."""
    return np.ascontiguousarray(
        arr.reshape(kc, 128, arr.shape[1]).transpose(1, 0, 2))


def kernel(x, y, r, W_e, W_r):
    nc = _get_program()
    bf = ml_dtypes.bfloat16
    f8 = ml_dtypes.float8_e4m3

    f_ab, f_r = _dft_bases()
    fab_p = _pmajor(f_ab, 4)
    fr_p = _pmajor(f_r, 4)

    wrT = np.zeros((R_PAD, D), dtype=bf)
    wrT[:R, :] = W_r.astype(bf).T
    wrT_p = _pmajor(wrT, RC)
    rT_pad = np.zeros((R_PAD, B), dtype=bf)
    rT_pad[:R, :] = np.ascontiguousarray(r.T).astype(bf)

    xT = np.ascontiguousarray(x.T).astype(f8)     # (E, B)
    yT = np.ascontiguousarray(y.T).astype(f8)
    weT = (np.ascontiguousarray(W_e.T) * S_WE).astype(f8)  # (E, D)

    in_maps = []
    for c in range(NCORES):
        lo, hi = c * E_SH, (c + 1) * E_SH
        xT_sh = np.zeros((E_PAD, B), dtype=f8)
        xT_sh[:E_SH] = xT[lo:hi]
        yT_sh = np.zeros((E_PAD, B), dtype=f8)
        yT_sh[:E_SH] = yT[lo:hi]
        weT_sh = np.zeros((E_PAD, D), dtype=f8)
        weT_sh[:E_SH] = weT[lo:hi]
        xp = _pmajor(xT_sh, KC)     # (128, KC, B)
        yp = _pmajor(yT_sh, KC)
        m = {
            "weT": _pmajor(weT_sh, KC),
            "rT": _pmajor(np.ascontiguousarray(rT_pad[:, _core_rows(c)]),
                          RC),
            "wrT": wrT_p,
            "fab": fab_p,
            "fr": fr_p,
        }
        for ci, (col0, ncols) in enumerate(CHUNKS):
            m[f"x{ci}"] = np.ascontiguousarray(xp[:, :, col0:col0 + ncols])
            m[f"y{ci}"] = np.ascontiguousarray(yp[:, :, col0:col0 + ncols])
        in_maps.append(m)

    res = run_bass_kernel_spmd(nc, in_maps, core_ids=list(range(NCORES)))
    out = np.empty((B, 1), dtype=np.float32)
    for c in range(NCORES):
        out[_core_rows(c)] = res.results[c]["out"]
    return out


# revision 52
# speedup vs baseline: 1.1599x; 1.1599x over previous
"""HolE scorer kernel for 8 Trainium2 NeuronCores (Bass/Tile).

Computation (reference):
    a = x @ W_e.T; b = y @ W_e.T; rr = r @ W_r.T          # (B, d)
    corr = irfft(rfft(a) * conj(rfft(b))) / d             # circular correlation
    out = sigmoid(sum(rr * corr, axis=1))                 # (B, 1)

Strategy:
  - Tensor-parallel over entities for the two big GEMMs: core c holds
    entity columns [c*12500, (c+1)*12500) of x, y, W_e (padded to 12544 =
    98*128), computing partial a.T/b.T (d-major) in fp8 e4m3 with
    DoubleRow perf mode (two 128-K tiles per instruction at 2x bf16 rate).
    W_e is pre-scaled by 2^14 into fp8 normal range; the inverse scale is
    folded into the a/b DFT basis (exact power-of-2).  Validated max rel
    err ~1.6e-2 on the final sigmoid output (tolerance 2e-2).
  - Batch columns are processed in 4 passes: y[0:512], y[512:1024],
    x[0:512], x[512:1024].  Each pass ends in a ReduceScatter(add)
    handing each core its 64 columns of fully-summed output; every RS
    except the final one hides under the next pass's GEMM.
  - Core c owns batch rows {c*64..+63, 512+c*64..+63} (local rows
    0:64, 64:128) — host gathers accordingly.
  - Tail per core (128 batch rows): rr.T GEMM, rfft via DFT-basis matmuls,
    and the irfft+rowwise-dot folded into a frequency-domain weighted dot
    (Parseval):  score_i = (1/d^2) sum_f w_f (Rr*Pr + Ri*Pi)[i,f],
    with P = A * conj(B), w = [1, 2, ..., 2, 1].  The w/d^2 factor is
    folded into the rr DFT basis; the rfft/score of each a-chunk runs as
    soon as that chunk's RS lands, overlapping the remaining GEMM passes.
  - All DRAM operands are staged host-side in partition-major layout
    (128, k-chunks, cols) so every DMA descriptor is a contiguous
    multi-KB run per partition.
  - Queue split: W_e/static/staging/tail DMAs ride the Scalar HWDGE
    queue, the streamed x/y tiles the Sync queue.
"""

import numpy as np
import ml_dtypes

import concourse.bass as bass
import concourse.tile as tile
from concourse import bacc, mybir
from concourse.alu_op_type import AluOpType
from concourse.bass_utils import run_bass_kernel_spmd

# Problem shapes (hardcoded per contract)
B = 1024            # batch
D = 512             # num_dim
E = 100000          # num_entities
R = 1000            # num_relations
NCORES = 8

E_SH = E // NCORES          # 12500 entities per core
KC = 98                     # k-chunks of 128 after padding (98*128 = 12544)
E_PAD = KC * 128            # 12544
KG = 7                      # k-groups
KJ = KC // KG               # 14 chunks per group
RC = 8                      # relation k-chunks (1000 -> 1024)
R_PAD = RC * 128
NF = D // 2 + 1             # 257 rfft bins
B_SH = B // NCORES          # 128 batch rows per core

# batch-column chunks: (col0, ncols); per-core share w = ncols // 8.
CHUNKS = [(0, 512), (512, 512)]
ROW_OFF = [0, 64]

BF16 = mybir.dt.bfloat16
F32 = mybir.dt.float32
F8 = mybir.dt.float8e4
S_WE = 16384.0              # 2^14: lifts W_e (~3e-3) into fp8 normal range

_cached = {}


def _dft_bases():
    d = D
    dd = np.arange(d, dtype=np.float64)[:, None]
    ff = np.arange(NF, dtype=np.float64)[None, :]
    ang = 2.0 * np.pi * dd * ff / d
    fr = np.cos(ang)
    fi = -np.sin(ang)
    # a/b arrive scaled by S_WE (fp8 W_e pre-scale); fold 1/S_WE here.
    f_ab = np.concatenate([fr, fi], axis=1) / S_WE       # (512, 514)
    w = np.full(NF, 2.0); w[0] = 1.0; w[-1] = 1.0
    scale = w / (d * d)
    f_r = np.concatenate([fr * scale, fi * scale], axis=1)
    return (f_ab.astype(ml_dtypes.bfloat16), f_r.astype(ml_dtypes.bfloat16))


def _build_program():
    nc = bacc.Bacc("TRN2", target_bir_lowering=False, debug=False,
                   num_devices=NCORES)

    # partition-major DRAM operands: (128, k-chunks, cols)
    x_d = [nc.dram_tensor(f"x{ci}", (128, KC, nc_), F8, kind="ExternalInput")
           for ci, (_, nc_) in enumerate(CHUNKS)]
    y_d = [nc.dram_tensor(f"y{ci}", (128, KC, nc_), F8, kind="ExternalInput")
           for ci, (_, nc_) in enumerate(CHUNKS)]
    weT_d = nc.dram_tensor("weT", (128, KC, D), F8, kind="ExternalInput")
    rT_d = nc.dram_tensor("rT", (128, RC, B_SH), BF16, kind="ExternalInput")
    wrT_d = nc.dram_tensor("wrT", (128, RC, D), BF16, kind="ExternalInput")
    fab_d = nc.dram_tensor("fab", (128, 4, 2 * NF), BF16, kind="ExternalInput")
    fr_d = nc.dram_tensor("fr", (128, 4, 2 * NF), BF16, kind="ExternalInput")
    out_d = nc.dram_tensor("out", (B_SH, 1), F32, kind="ExternalOutput")

    # staging + reduce-scatter outputs.  One 512KB mesh per b pass (small
    # payloads keep worst-case mesh spans down, and both are triggered
    # 100us+ before their results are consumed in the tail); the last
    # pass is reduce-scattered in two K-stages ("a1A"/"a1B").
    stages = {}
    rs_outs = {}
    for key in ("b0", "b1", "a0", "a1A", "a1B"):
        stages[key] = nc.dram_tensor(f"stage_{key}", (NCORES, D, 64), BF16)
        rs_outs[key] = nc.dram_tensor(f"rs_{key}", (D, 64), BF16)
    groups = [list(range(NCORES))]

    with tile.TileContext(nc) as tc:
        with (
            tc.tile_pool(name="weights", bufs=1) as wpool,
            tc.tile_pool(name="stream", bufs=7) as spool,
            tc.tile_pool(name="copies", bufs=4) as cpool,
            tc.tile_pool(name="tail", bufs=1) as tpool,
            tc.tile_pool(name="psum", bufs=8, space="PSUM") as ppool,
        ):
            # ---- resident W_e.T groups (Scalar queue, we0 gated fine) ----
            we_tiles = []
            for g in range(KG):
                wt = wpool.tile([128, KJ, D], F8, tag=f"we{g}", name=f"we{g}")
                src = weT_d[:, g * KJ:(g + 1) * KJ, :]
                if g == 0:
                    nc.scalar.dma_start(wt[:, 0:2], src[:, 0:2])
                    nc.scalar.dma_start(wt[:, 2:4], src[:, 2:4])
                    nc.scalar.dma_start(wt[:, 4:8], src[:, 4:8])
                    nc.scalar.dma_start(wt[:, 8:KJ], src[:, 8:KJ])
                else:
                    nc.scalar.dma_start(wt[:], src)
                we_tiles.append(wt)

            # small static tensors, needed only mid-kernel (Scalar queue)
            r_t = wpool.tile([128, RC, B_SH], BF16, tag="r", name="r")
            nc.scalar.dma_start(r_t[:], rT_d[:])
            wr_t = wpool.tile([128, RC, D], BF16, tag="wr", name="wr")
            nc.scalar.dma_start(wr_t[:], wrT_d[:])
            fab_t = wpool.tile([128, 4, 2 * NF], BF16, tag="fab", name="fab")
            nc.scalar.dma_start(fab_t[:], fab_d[:])
            fr_t = wpool.tile([128, 4, 2 * NF], BF16, tag="frq", name="frq")
            nc.scalar.dma_start(fr_t[:], fr_d[:])

            rr_b = tpool.tile([128, 4, B_SH], BF16, name="rr_b")

            def load_chunk(key, w_=64):
                tb = tpool.tile([128, 4, w_], BF16, name=f"ld_{key}")
                nc.scalar.dma_start(
                    tb[:],
                    rs_outs[key][:].rearrange("(mc p) q -> p mc q", p=128))
                return tb

            def rfft_mm(src_b, basis, psr, psi, lo, w_,
                        start=True, stop=True):
                for k in range(4):
                    nc.tensor.matmul(psr[lo:lo + w_, :], src_b[:, k, :],
                                     basis[:, k, 0:NF],
                                     start=(start and k == 0),
                                     stop=(stop and k == 3))
                for k in range(4):
                    nc.tensor.matmul(psi[lo:lo + w_, :], src_b[:, k, :],
                                     basis[:, k, NF:2 * NF],
                                     start=(start and k == 0),
                                     stop=(stop and k == 3))

            f1 = tpool.tile([B_SH, NF], F32, name="f1")
            f2 = tpool.tile([B_SH, NF], F32, name="f2")
            g_t = tpool.tile([B_SH, 2 * NF], F32, name="g_t")
            sig = tpool.tile([B_SH, 1], F32, name="sig")
            ps_ar = None
            ps_ai = None
            a_tiles = {}

            def a_score_tail(ci, tiles):
                """rfft chunk ci of a (from partial-sum tiles, summed via
                PSUM accumulation), combine with F1/F2, score+sigmoid."""
                lo = ROW_OFF[ci]
                for ti, a_tile in enumerate(tiles):
                    rfft_mm(a_tile, fab_t, ps_ar, ps_ai, lo, 64,
                            start=(ti == 0), stop=(ti == len(tiles) - 1))
                sl = slice(lo, lo + 64)
                nc.vector.tensor_tensor(g_t[sl, 0:NF], ps_ar[sl], f1[sl],
                                        AluOpType.mult)
                nc.vector.tensor_tensor(g_t[sl, NF:2 * NF], ps_ai[sl],
                                        f2[sl], AluOpType.mult)
                score = tpool.tile([64, 1], F32, tag="score",
                                   name=f"score{ci}")
                nc.vector.reduce_sum(score[:], g_t[sl, :],
                                     axis=mybir.AxisListType.X)
                nc.scalar.activation(sig[sl], score[:],
                                     mybir.ActivationFunctionType.Sigmoid)
                nc.sync.dma_start(out_d[sl, :], sig[sl])

            def emit_gemm(mat_d, accs, g_lo, g_hi, kp_lo, kp_hi, nm,
                          gate_first=False):
                for g in range(g_lo, g_hi):
                    xt = spool.tile([128, KJ, 512], F8, tag="xs",
                                    name=f"xs{nm}{g}")
                    src = mat_d[:, g * KJ:(g + 1) * KJ, :]
                    if gate_first and g == 0:
                        nc.sync.dma_start(xt[:, 0:2], src[:, 0:2])
                        nc.sync.dma_start(xt[:, 2:4], src[:, 2:4])
                        nc.sync.dma_start(xt[:, 4:8], src[:, 4:8])
                        nc.sync.dma_start(xt[:, 8:KJ], src[:, 8:KJ])
                    else:
                        nc.sync.dma_start(xt[:], src)
                    for j2 in range(0, KJ, 2):
                        kp = (g * KJ + j2) // 2
                        for m in range(4):
                            nc.tensor.matmul(
                                accs[m][:],
                                we_tiles[g][:, j2:j2 + 2,
                                            m * 128:(m + 1) * 128],
                                xt[:, j2:j2 + 2, :],
                                start=(kp == kp_lo), stop=(kp == kp_hi),
                                perf_mode=mybir.MatmulPerfMode.DoubleRow)

            def emit_stage(accs, key, nm, q0=0, queues=None, order=None):
                # queues: per-m DMA-issuing engines; the partition->core
                # scatter emits many small descriptors, so spreading the
                # four m-tiles over multiple queues cuts its latency.
                # order: copy emission order — put the tiles bound for the
                # least-busy queue first so their DMAs start earliest.
                queues = queues or [nc.scalar] * 4
                for m in order or range(4):
                    sb = cpool.tile([128, 512], BF16, tag="cp",
                                    name=f"cp{nm}{m}")
                    nc.vector.tensor_copy(sb[:], accs[m][:])
                    dst = (stages[key][:, m * 128:(m + 1) * 128,
                                       q0:q0 + 64]
                           .rearrange("c d q -> d c q"))
                    src = sb.rearrange("d (c q) -> d c q", c=NCORES)
                    queues[m].dma_start(dst, src)

            def emit_rs(key):
                nc.gpsimd.collective_compute(
                    "ReduceScatter", AluOpType.add,
                    replica_groups=groups,
                    ins=[stages[key][:].opt()],
                    outs=[rs_outs[key][:].opt()])

            # ---- main GEMM passes ----
            passes = [("b", 0), ("b", 1), ("a", 0)]
            for pi_, (mat, ci) in enumerate(passes):
                mat_d = y_d[ci] if mat == "b" else x_d[ci]
                key = f"{mat}{ci}"
                accs = [
                    ppool.tile([128, 512], F32, tag="acc",
                               name=f"acc{key}{m}")
                    for m in range(4)
                ]
                emit_gemm(mat_d, accs, 0, KG, 0, KC // 2 - 1, key,
                          gate_first=(pi_ == 0))
                emit_stage(accs, key, key)
                emit_rs(key)

                if pi_ == 0:
                    # rr.T GEMM slotted after the first pass: its inputs are
                    # small and arrive behind that pass's stream DMAs.
                    ps_rr = ppool.tile([128, 4, B_SH], F32, tag="acc",
                                       name="ps_rr")
                    for m in range(4):
                        for j in range(RC):
                            nc.tensor.matmul(
                                ps_rr[:, m, :],
                                wr_t[:, j, m * 128:(m + 1) * 128],
                                r_t[:, j, :],
                                start=(j == 0), stop=(j == RC - 1))
                    nc.vector.tensor_copy(rr_b[:], ps_rr[:])

                if pi_ == 2:
                    # prefetch the b rs-outputs now: they sit on the scalar
                    # FIFO after this pass's staging, so they fire as soon
                    # as their meshes end without blocking anything.
                    b_tiles = [load_chunk("b0"), load_chunk("b1")]

            # ---- last pass a[512:1024], reduce-scattered in two K-stages:
            # stage A (k-pairs 0..20) triggers its RS mid-pass so the mesh
            # latency and inter-core skew are absorbed under stage B's GEMM
            # (k-pairs 21..48); the final RS then starts with cores already
            # aligned and only ~12us of mesh is exposed.  The two partial
            # sums recombine for free inside the rfft's PSUM accumulation.
            KPA = 3 * KJ // 2 - 1                 # last k-pair of stage A
            # allocate both stages' PSUM up front: accsB lands on banks
            # freed at the a0 pass's end, so stage B's GEMM never waits on
            # stage A's copies.
            accsA = [ppool.tile([128, 512], F32, tag="acc",
                                name=f"accA{m}") for m in range(4)]
            accsB = [ppool.tile([128, 512], F32, tag="acc",
                                name=f"accB{m}") for m in range(4)]
            emit_gemm(x_d[1], accsA, 0, 3, 0, KPA, "a1A")
            # stage B's stream tiles: issued now (behind stage A's on the
            # sync queue), kept resident — stage B's GEMM runs m-major and
            # re-reads each tile four times.
            xtB = []
            for g in range(3, KG):
                xt = spool.tile([128, KJ, 512], F8, tag="xs",
                                name=f"xsa1B{g}")
                nc.sync.dma_start(xt[:], x_d[1][:, g * KJ:(g + 1) * KJ, :])
                xtB.append(xt)
            emit_stage(accsA, "a1A", "a1A")
            emit_rs("a1A")

            # stage B GEMM, m-major: each m-tile's accumulation finishes a
            # quarter of the way through, so its copy + staging scatter
            # overlap the remaining GEMM — only m3's staging (~5us) is
            # left between the last matmul and the final mesh trigger.
            bq = [nc.scalar, nc.sync, nc.scalar, nc.sync]
            for m in range(4):
                for g in range(3, KG):
                    for j2 in range(0, KJ, 2):
                        kp = (g * KJ + j2) // 2
                        nc.tensor.matmul(
                            accsB[m][:],
                            we_tiles[g][:, j2:j2 + 2,
                                        m * 128:(m + 1) * 128],
                            xtB[g - 3][:, j2:j2 + 2, :],
                            start=(kp == KPA + 1), stop=(kp == KC // 2 - 1),
                            perf_mode=mybir.MatmulPerfMode.DoubleRow)
                sb = cpool.tile([128, 512], BF16, tag="cp", name=f"cpB{m}")
                nc.vector.tensor_copy(sb[:], accsB[m][:])
                dst = (stages["a1B"][:, m * 128:(m + 1) * 128, :]
                       .rearrange("c d q -> d c q"))
                bq[m].dma_start(dst, sb.rearrange("d (c q) -> d c q",
                                                  c=NCORES))
            emit_rs("a1B")
            a_tiles[0] = load_chunk("a0")
            a_tiles[1] = load_chunk("a1A")
            a_tiles[2] = load_chunk("a1B")

            # ---- tail: all spectrum work overlaps the exposed final
            # mesh.  By now every earlier mesh has had 90us+ to complete,
            # so even pathological mesh spans cannot stall the GEMMs.
            # F1 = Rr.Br - Ri.Bi, F2 = Rr.Bi + Ri.Br.
            ps_br = ppool.tile([B_SH, NF], F32, tag="acc", name="ps_br")
            ps_bi = ppool.tile([B_SH, NF], F32, tag="acc", name="ps_bi")
            ps_qr = ppool.tile([B_SH, NF], F32, tag="acc", name="ps_qr")
            ps_qi = ppool.tile([B_SH, NF], F32, tag="acc", name="ps_qi")
            for bci in range(2):
                rfft_mm(b_tiles[bci], fab_t, ps_br, ps_bi,
                        ROW_OFF[bci], 64)
            rfft_mm(rr_b, fr_t, ps_qr, ps_qi, 0, B_SH)
            s_qr = tpool.tile([B_SH, NF], F32, name="s_qr")
            nc.vector.tensor_copy(s_qr[:], ps_qr[:])
            s_qi = tpool.tile([B_SH, NF], F32, name="s_qi")
            nc.vector.tensor_copy(s_qi[:], ps_qi[:])
            t1 = tpool.tile([B_SH, NF], F32, name="t1")
            t2 = tpool.tile([B_SH, NF], F32, name="t2")
            nc.vector.tensor_tensor(f1[:], ps_br[:], s_qr[:],
                                    AluOpType.mult)
            nc.vector.tensor_tensor(t1[:], ps_bi[:], s_qi[:],
                                    AluOpType.mult)
            nc.vector.tensor_tensor(f1[:], f1[:], t1[:],
                                    AluOpType.subtract)
            nc.vector.tensor_tensor(f2[:], ps_bi[:], s_qr[:],
                                    AluOpType.mult)
            nc.vector.tensor_tensor(t2[:], ps_br[:], s_qi[:],
                                    AluOpType.mult)
            nc.vector.tensor_tensor(f2[:], f2[:], t2[:],
                                    AluOpType.add)

            ps_ar = ppool.tile([B_SH, NF], F32, tag="acc", name="ps_ar")
            ps_ai = ppool.tile([B_SH, NF], F32, tag="acc", name="ps_ai")
            a_score_tail(0, [a_tiles[0]])
            a_score_tail(1, [a_tiles[1], a_tiles[2]])

    nc.compile()
    return nc


def _get_program():
    if "nc" not in _cached:
        _cached["nc"] = _build_program()
    return _cached["nc"]


def _core_rows(c):
    """Batch rows owned by core c (matches CHUNKS/ROW_OFF mapping)."""
    return np.r_[c * 64:(c + 1) * 64,
                 512 + c * 64:512 + (c + 1) * 64]


def _pmajor(arr, kc):
    """(kc*128, cols) -> partition-major (128, kc, cols), contiguous."""
    return np.ascontiguousarray(
        arr.reshape(kc, 128, arr.shape[1]).transpose(1, 0, 2))


def kernel(x, y, r, W_e, W_r):
    nc = _get_program()
    bf = ml_dtypes.bfloat16
    f8 = ml_dtypes.float8_e4m3

    f_ab, f_r = _dft_bases()
    fab_p = _pmajor(f_ab, 4)
    fr_p = _pmajor(f_r, 4)

    wrT = np.zeros((R_PAD, D), dtype=bf)
    wrT[:R, :] = W_r.astype(bf).T
    wrT_p = _pmajor(wrT, RC)
    rT_pad = np.zeros((R_PAD, B), dtype=bf)
    rT_pad[:R, :] = np.ascontiguousarray(r.T).astype(bf)

    xT = np.ascontiguousarray(x.T).astype(f8)     # (E, B)
    yT = np.ascontiguousarray(y.T).astype(f8)
    weT = (np.ascontiguousarray(W_e.T) * S_WE).astype(f8)  # (E, D)

    in_maps = []
    for c in range(NCORES):
        lo, hi = c * E_SH, (c + 1) * E_SH
        xT_sh = np.zeros((E_PAD, B), dtype=f8)
        xT_sh[:E_SH] = xT[lo:hi]
        yT_sh = np.zeros((E_PAD, B), dtype=f8)
        yT_sh[:E_SH] = yT[lo:hi]
        weT_sh = np.zeros((E_PAD, D), dtype=f8)
        weT_sh[:E_SH] = weT[lo:hi]
        xp = _pmajor(xT_sh, KC)     # (128, KC, B)
        yp = _pmajor(yT_sh, KC)
        m = {
            "weT": _pmajor(weT_sh, KC),
            "rT": _pmajor(np.ascontiguousarray(rT_pad[:, _core_rows(c)]),
                          RC),
            "wrT": wrT_p,
            "fab": fab_p,
            "fr": fr_p,
        }
        for ci, (col0, ncols) in enumerate(CHUNKS):
            m[f"x{ci}"] = np.ascontiguousarray(xp[:, :, col0:col0 + ncols])
            m[f"y{ci}"] = np.ascontiguousarray(yp[:, :, col0:col0 + ncols])
        in_maps.append(m)

    res = run_bass_kernel_spmd(nc, in_maps, core_ids=list(range(NCORES)))
    out = np.empty((B, 1), dtype=np.float32)
    for c in range(NCORES):
        out[_core_rows(c)] = res.results[c]["out"]
    return out


# revision 53
# speedup vs baseline: 1.1709x; 1.0095x over previous
"""HolE scorer kernel for 8 Trainium2 NeuronCores (Bass/Tile).

Computation (reference):
    a = x @ W_e.T; b = y @ W_e.T; rr = r @ W_r.T          # (B, d)
    corr = irfft(rfft(a) * conj(rfft(b))) / d             # circular correlation
    out = sigmoid(sum(rr * corr, axis=1))                 # (B, 1)

Strategy:
  - Tensor-parallel over entities for the two big GEMMs: core c holds
    entity columns [c*12500, (c+1)*12500) of x, y, W_e (padded to 12544 =
    98*128), computing partial a.T/b.T (d-major) in fp8 e4m3 with
    DoubleRow perf mode (two 128-K tiles per instruction at 2x bf16 rate).
    W_e is pre-scaled by 2^14 into fp8 normal range; the inverse scale is
    folded into the a/b DFT basis (exact power-of-2).  Validated max rel
    err ~1.6e-2 on the final sigmoid output (tolerance 2e-2).
  - Batch columns are processed in 4 passes: y[0:512], y[512:1024],
    x[0:512], x[512:1024].  Each pass ends in a ReduceScatter(add)
    handing each core its 64 columns of fully-summed output; every RS
    except the final one hides under the next pass's GEMM.
  - Core c owns batch rows {c*64..+63, 512+c*64..+63} (local rows
    0:64, 64:128) — host gathers accordingly.
  - Tail per core (128 batch rows): rr.T GEMM, rfft via DFT-basis matmuls,
    and the irfft+rowwise-dot folded into a frequency-domain weighted dot
    (Parseval):  score_i = (1/d^2) sum_f w_f (Rr*Pr + Ri*Pi)[i,f],
    with P = A * conj(B), w = [1, 2, ..., 2, 1].  The w/d^2 factor is
    folded into the rr DFT basis; the rfft/score of each a-chunk runs as
    soon as that chunk's RS lands, overlapping the remaining GEMM passes.
  - All DRAM operands are staged host-side in partition-major layout
    (128, k-chunks, cols) so every DMA descriptor is a contiguous
    multi-KB run per partition.
  - Queue split: W_e/static/staging/tail DMAs ride the Scalar HWDGE
    queue, the streamed x/y tiles the Sync queue.
"""

import numpy as np
import ml_dtypes

import concourse.bass as bass
import concourse.tile as tile
from concourse import bacc, mybir
from concourse.alu_op_type import AluOpType
from concourse.bass_utils import run_bass_kernel_spmd

# Problem shapes (hardcoded per contract)
B = 1024            # batch
D = 512             # num_dim
E = 100000          # num_entities
R = 1000            # num_relations
NCORES = 8

E_SH = E // NCORES          # 12500 entities per core
KC = 98                     # k-chunks of 128 after padding (98*128 = 12544)
E_PAD = KC * 128            # 12544
KG = 7                      # k-groups
KJ = KC // KG               # 14 chunks per group
RC = 8                      # relation k-chunks (1000 -> 1024)
R_PAD = RC * 128
NF = D // 2 + 1             # 257 rfft bins
B_SH = B // NCORES          # 128 batch rows per core

# batch-column chunks: (col0, ncols); per-core share w = ncols // 8.
CHUNKS = [(0, 512), (512, 512)]
ROW_OFF = [0, 64]

BF16 = mybir.dt.bfloat16
F32 = mybir.dt.float32
F8 = mybir.dt.float8e4
S_WE = 16384.0              # 2^14: lifts W_e (~3e-3) into fp8 normal range

_cached = {}


def _dft_bases():
    d = D
    dd = np.arange(d, dtype=np.float64)[:, None]
    ff = np.arange(NF, dtype=np.float64)[None, :]
    ang = 2.0 * np.pi * dd * ff / d
    fr = np.cos(ang)
    fi = -np.sin(ang)
    # a/b arrive scaled by S_WE (fp8 W_e pre-scale); fold 1/S_WE here.
    f_ab = np.concatenate([fr, fi], axis=1) / S_WE       # (512, 514)
    w = np.full(NF, 2.0); w[0] = 1.0; w[-1] = 1.0
    scale = w / (d * d)
    f_r = np.concatenate([fr * scale, fi * scale], axis=1)
    return (f_ab.astype(ml_dtypes.bfloat16), f_r.astype(ml_dtypes.bfloat16))


def _build_program():
    nc = bacc.Bacc("TRN2", target_bir_lowering=False, debug=False,
                   num_devices=NCORES)

    # partition-major DRAM operands: (128, k-chunks, cols)
    x_d = [nc.dram_tensor(f"x{ci}", (128, KC, nc_), F8, kind="ExternalInput")
           for ci, (_, nc_) in enumerate(CHUNKS)]
    y_d = [nc.dram_tensor(f"y{ci}", (128, KC, nc_), F8, kind="ExternalInput")
           for ci, (_, nc_) in enumerate(CHUNKS)]
    weT_d = nc.dram_tensor("weT", (128, KC, D), F8, kind="ExternalInput")
    rT_d = nc.dram_tensor("rT", (128, RC, B_SH), BF16, kind="ExternalInput")
    wrT_d = nc.dram_tensor("wrT", (128, RC, D), BF16, kind="ExternalInput")
    fab_d = nc.dram_tensor("fab", (128, 4, 2 * NF), BF16, kind="ExternalInput")
    fr_d = nc.dram_tensor("fr", (128, 4, 2 * NF), BF16, kind="ExternalInput")
    out_d = nc.dram_tensor("out", (B_SH, 1), F32, kind="ExternalOutput")

    # staging + reduce-scatter outputs.  One 512KB mesh per b pass (small
    # payloads keep worst-case mesh spans down, and both are triggered
    # 100us+ before their results are consumed in the tail); the last
    # pass is reduce-scattered in two K-stages ("a1A"/"a1B").
    stages = {}
    rs_outs = {}
    for key in ("b0", "b1", "a0", "a1A", "a1B"):
        stages[key] = nc.dram_tensor(f"stage_{key}", (NCORES, D, 64), BF16)
        rs_outs[key] = nc.dram_tensor(f"rs_{key}", (D, 64), BF16)
    groups = [list(range(NCORES))]

    with tile.TileContext(nc) as tc:
        with (
            tc.tile_pool(name="weights", bufs=1) as wpool,
            tc.tile_pool(name="stream", bufs=7) as spool,
            tc.tile_pool(name="copies", bufs=4) as cpool,
            tc.tile_pool(name="tail", bufs=1) as tpool,
            tc.tile_pool(name="psum", bufs=8, space="PSUM") as ppool,
        ):
            # ---- resident W_e.T groups (Scalar queue, we0 gated fine) ----
            we_tiles = []
            for g in range(KG):
                wt = wpool.tile([128, KJ, D], F8, tag=f"we{g}", name=f"we{g}")
                src = weT_d[:, g * KJ:(g + 1) * KJ, :]
                if g == 0:
                    nc.scalar.dma_start(wt[:, 0:2], src[:, 0:2])
                    nc.scalar.dma_start(wt[:, 2:4], src[:, 2:4])
                    nc.scalar.dma_start(wt[:, 4:8], src[:, 4:8])
                    nc.scalar.dma_start(wt[:, 8:KJ], src[:, 8:KJ])
                else:
                    nc.scalar.dma_start(wt[:], src)
                we_tiles.append(wt)

            # small static tensors, needed only mid-kernel (Scalar queue)
            r_t = wpool.tile([128, RC, B_SH], BF16, tag="r", name="r")
            nc.scalar.dma_start(r_t[:], rT_d[:])
            wr_t = wpool.tile([128, RC, D], BF16, tag="wr", name="wr")
            nc.scalar.dma_start(wr_t[:], wrT_d[:])
            fab_t = wpool.tile([128, 4, 2 * NF], BF16, tag="fab", name="fab")
            nc.scalar.dma_start(fab_t[:], fab_d[:])
            fr_t = wpool.tile([128, 4, 2 * NF], BF16, tag="frq", name="frq")
            nc.scalar.dma_start(fr_t[:], fr_d[:])

            rr_b = tpool.tile([128, 4, B_SH], BF16, name="rr_b")

            def load_chunk(key, w_=64):
                tb = tpool.tile([128, 4, w_], BF16, name=f"ld_{key}")
                nc.scalar.dma_start(
                    tb[:],
                    rs_outs[key][:].rearrange("(mc p) q -> p mc q", p=128))
                return tb

            def rfft_mm(src_b, basis, psr, psi, lo, w_,
                        start=True, stop=True):
                for k in range(4):
                    nc.tensor.matmul(psr[lo:lo + w_, :], src_b[:, k, :],
                                     basis[:, k, 0:NF],
                                     start=(start and k == 0),
                                     stop=(stop and k == 3))
                for k in range(4):
                    nc.tensor.matmul(psi[lo:lo + w_, :], src_b[:, k, :],
                                     basis[:, k, NF:2 * NF],
                                     start=(start and k == 0),
                                     stop=(stop and k == 3))

            f1 = tpool.tile([B_SH, NF], F32, name="f1")
            f2 = tpool.tile([B_SH, NF], F32, name="f2")
            g_t = tpool.tile([B_SH, 2 * NF], F32, name="g_t")
            sig = tpool.tile([B_SH, 1], F32, name="sig")
            ps_ar = None
            ps_ai = None
            a_tiles = {}

            def a_score_tail(ci, tiles):
                """rfft chunk ci of a (from partial-sum tiles, summed via
                PSUM accumulation), combine with F1/F2, score+sigmoid."""
                lo = ROW_OFF[ci]
                for ti, a_tile in enumerate(tiles):
                    rfft_mm(a_tile, fab_t, ps_ar, ps_ai, lo, 64,
                            start=(ti == 0), stop=(ti == len(tiles) - 1))
                sl = slice(lo, lo + 64)
                nc.vector.tensor_tensor(g_t[sl, 0:NF], ps_ar[sl], f1[sl],
                                        AluOpType.mult)
                nc.vector.tensor_tensor(g_t[sl, NF:2 * NF], ps_ai[sl],
                                        f2[sl], AluOpType.mult)
                score = tpool.tile([64, 1], F32, tag="score",
                                   name=f"score{ci}")
                nc.vector.reduce_sum(score[:], g_t[sl, :],
                                     axis=mybir.AxisListType.X)
                nc.scalar.activation(sig[sl], score[:],
                                     mybir.ActivationFunctionType.Sigmoid)
                nc.sync.dma_start(out_d[sl, :], sig[sl])

            def emit_gemm(mat_d, accs, g_lo, g_hi, kp_lo, kp_hi, nm,
                          gate_first=False):
                for g in range(g_lo, g_hi):
                    xt = spool.tile([128, KJ, 512], F8, tag="xs",
                                    name=f"xs{nm}{g}")
                    src = mat_d[:, g * KJ:(g + 1) * KJ, :]
                    if gate_first and g == 0:
                        nc.sync.dma_start(xt[:, 0:2], src[:, 0:2])
                        nc.sync.dma_start(xt[:, 2:4], src[:, 2:4])
                        nc.sync.dma_start(xt[:, 4:8], src[:, 4:8])
                        nc.sync.dma_start(xt[:, 8:KJ], src[:, 8:KJ])
                    elif gate_first and g == 1:
                        # half-gate g1 as well: the PE reaches it before the
                        # whole tile can land behind g0's stream
                        half = KJ // 2
                        nc.sync.dma_start(xt[:, :half], src[:, :half])
                        nc.sync.dma_start(xt[:, half:], src[:, half:])
                    else:
                        nc.sync.dma_start(xt[:], src)
                    for j2 in range(0, KJ, 2):
                        kp = (g * KJ + j2) // 2
                        for m in range(4):
                            nc.tensor.matmul(
                                accs[m][:],
                                we_tiles[g][:, j2:j2 + 2,
                                            m * 128:(m + 1) * 128],
                                xt[:, j2:j2 + 2, :],
                                start=(kp == kp_lo), stop=(kp == kp_hi),
                                perf_mode=mybir.MatmulPerfMode.DoubleRow)

            def emit_stage(accs, key, nm, q0=0, queues=None, order=None):
                # queues: per-m DMA-issuing engines; the partition->core
                # scatter emits many small descriptors, so spreading the
                # four m-tiles over multiple queues cuts its latency.
                # order: copy emission order — put the tiles bound for the
                # least-busy queue first so their DMAs start earliest.
                queues = queues or [nc.scalar] * 4
                for m in order or range(4):
                    sb = cpool.tile([128, 512], BF16, tag="cp",
                                    name=f"cp{nm}{m}")
                    nc.vector.tensor_copy(sb[:], accs[m][:])
                    dst = (stages[key][:, m * 128:(m + 1) * 128,
                                       q0:q0 + 64]
                           .rearrange("c d q -> d c q"))
                    src = sb.rearrange("d (c q) -> d c q", c=NCORES)
                    queues[m].dma_start(dst, src)

            def emit_rs(key):
                nc.gpsimd.collective_compute(
                    "ReduceScatter", AluOpType.add,
                    replica_groups=groups,
                    ins=[stages[key][:].opt()],
                    outs=[rs_outs[key][:].opt()])

            # ---- main GEMM passes ----
            passes = [("b", 0), ("b", 1), ("a", 0)]
            for pi_, (mat, ci) in enumerate(passes):
                mat_d = y_d[ci] if mat == "b" else x_d[ci]
                key = f"{mat}{ci}"
                accs = [
                    ppool.tile([128, 512], F32, tag="acc",
                               name=f"acc{key}{m}")
                    for m in range(4)
                ]
                emit_gemm(mat_d, accs, 0, KG, 0, KC // 2 - 1, key,
                          gate_first=(pi_ == 0))
                emit_stage(accs, key, key)
                emit_rs(key)

                if pi_ == 0:
                    # rr.T GEMM slotted after the first pass: its inputs are
                    # small and arrive behind that pass's stream DMAs.
                    ps_rr = ppool.tile([128, 4, B_SH], F32, tag="acc",
                                       name="ps_rr")
                    for m in range(4):
                        for j in range(RC):
                            nc.tensor.matmul(
                                ps_rr[:, m, :],
                                wr_t[:, j, m * 128:(m + 1) * 128],
                                r_t[:, j, :],
                                start=(j == 0), stop=(j == RC - 1))
                    nc.vector.tensor_copy(rr_b[:], ps_rr[:])

                if pi_ == 2:
                    # prefetch the b rs-outputs now: they sit on the scalar
                    # FIFO after this pass's staging, so they fire as soon
                    # as their meshes end without blocking anything.
                    b_tiles = [load_chunk("b0"), load_chunk("b1")]

            # ---- last pass a[512:1024], reduce-scattered in two K-stages:
            # stage A (k-pairs 0..20) triggers its RS mid-pass so the mesh
            # latency and inter-core skew are absorbed under stage B's GEMM
            # (k-pairs 21..48); the final RS then starts with cores already
            # aligned and only ~12us of mesh is exposed.  The two partial
            # sums recombine for free inside the rfft's PSUM accumulation.
            KPA = 3 * KJ // 2 - 1                 # last k-pair of stage A
            # allocate both stages' PSUM up front: accsB lands on banks
            # freed at the a0 pass's end, so stage B's GEMM never waits on
            # stage A's copies.
            accsA = [ppool.tile([128, 512], F32, tag="acc",
                                name=f"accA{m}") for m in range(4)]
            accsB = [ppool.tile([128, 512], F32, tag="acc",
                                name=f"accB{m}") for m in range(4)]
            emit_gemm(x_d[1], accsA, 0, 3, 0, KPA, "a1A")
            # stage B's stream tiles: issued now (behind stage A's on the
            # sync queue), kept resident — stage B's GEMM runs m-major and
            # re-reads each tile four times.
            xtB = []
            for g in range(3, KG):
                xt = spool.tile([128, KJ, 512], F8, tag="xs",
                                name=f"xsa1B{g}")
                nc.sync.dma_start(xt[:], x_d[1][:, g * KJ:(g + 1) * KJ, :])
                xtB.append(xt)
            emit_stage(accsA, "a1A", "a1A")
            emit_rs("a1A")

            # stage B GEMM, m-major: each m-tile's accumulation finishes a
            # quarter of the way through, so its copy + staging scatter
            # overlap the remaining GEMM — only m3's staging (~5us) is
            # left between the last matmul and the final mesh trigger.
            bq = [nc.scalar, nc.sync, nc.scalar, nc.sync]
            for m in range(4):
                for g in range(3, KG):
                    for j2 in range(0, KJ, 2):
                        kp = (g * KJ + j2) // 2
                        nc.tensor.matmul(
                            accsB[m][:],
                            we_tiles[g][:, j2:j2 + 2,
                                        m * 128:(m + 1) * 128],
                            xtB[g - 3][:, j2:j2 + 2, :],
                            start=(kp == KPA + 1), stop=(kp == KC // 2 - 1),
                            perf_mode=mybir.MatmulPerfMode.DoubleRow)
                sb = cpool.tile([128, 512], BF16, tag="cp", name=f"cpB{m}")
                nc.vector.tensor_copy(sb[:], accsB[m][:])
                dst = (stages["a1B"][:, m * 128:(m + 1) * 128, :]
                       .rearrange("c d q -> d c q"))
                bq[m].dma_start(dst, sb.rearrange("d (c q) -> d c q",
                                                  c=NCORES))
            emit_rs("a1B")
            a_tiles[0] = load_chunk("a0")
            a_tiles[1] = load_chunk("a1A")
            a_tiles[2] = load_chunk("a1B")

            # ---- tail: all spectrum work overlaps the exposed final
            # mesh.  By now every earlier mesh has had 90us+ to complete,
            # so even pathological mesh spans cannot stall the GEMMs.
            # F1 = Rr.Br - Ri.Bi, F2 = Rr.Bi + Ri.Br.
            ps_br = ppool.tile([B_SH, NF], F32, tag="acc", name="ps_br")
            ps_bi = ppool.tile([B_SH, NF], F32, tag="acc", name="ps_bi")
            ps_qr = ppool.tile([B_SH, NF], F32, tag="acc", name="ps_qr")
            ps_qi = ppool.tile([B_SH, NF], F32, tag="acc", name="ps_qi")
            for bci in range(2):
                rfft_mm(b_tiles[bci], fab_t, ps_br, ps_bi,
                        ROW_OFF[bci], 64)
            rfft_mm(rr_b, fr_t, ps_qr, ps_qi, 0, B_SH)
            s_qr = tpool.tile([B_SH, NF], F32, name="s_qr")
            nc.vector.tensor_copy(s_qr[:], ps_qr[:])
            s_qi = tpool.tile([B_SH, NF], F32, name="s_qi")
            nc.vector.tensor_copy(s_qi[:], ps_qi[:])
            t1 = tpool.tile([B_SH, NF], F32, name="t1")
            t2 = tpool.tile([B_SH, NF], F32, name="t2")
            nc.vector.tensor_tensor(f1[:], ps_br[:], s_qr[:],
                                    AluOpType.mult)
            nc.vector.tensor_tensor(t1[:], ps_bi[:], s_qi[:],
                                    AluOpType.mult)
            nc.vector.tensor_tensor(f1[:], f1[:], t1[:],
                                    AluOpType.subtract)
            nc.vector.tensor_tensor(f2[:], ps_bi[:], s_qr[:],
                                    AluOpType.mult)
            nc.vector.tensor_tensor(t2[:], ps_br[:], s_qi[:],
                                    AluOpType.mult)
            nc.vector.tensor_tensor(f2[:], f2[:], t2[:],
                                    AluOpType.add)

            ps_ar = ppool.tile([B_SH, NF], F32, tag="acc", name="ps_ar")
            ps_ai = ppool.tile([B_SH, NF], F32, tag="acc", name="ps_ai")
            a_score_tail(0, [a_tiles[0]])
            a_score_tail(1, [a_tiles[1], a_tiles[2]])

    nc.compile()
    return nc


def _get_program():
    if "nc" not in _cached:
        _cached["nc"] = _build_program()
    return _cached["nc"]


def _core_rows(c):
    """Batch rows owned by core c (matches CHUNKS/ROW_OFF mapping)."""
    return np.r_[c * 64:(c + 1) * 64,
                 512 + c * 64:512 + (c + 1) * 64]


def _pmajor(arr, kc):
    """(kc*128, cols) -> partition-major (128, kc, cols), contiguous."""
    return np.ascontiguousarray(
        arr.reshape(kc, 128, arr.shape[1]).transpose(1, 0, 2))


def kernel(x, y, r, W_e, W_r):
    nc = _get_program()
    bf = ml_dtypes.bfloat16
    f8 = ml_dtypes.float8_e4m3

    f_ab, f_r = _dft_bases()
    fab_p = _pmajor(f_ab, 4)
    fr_p = _pmajor(f_r, 4)

    wrT = np.zeros((R_PAD, D), dtype=bf)
    wrT[:R, :] = W_r.astype(bf).T
    wrT_p = _pmajor(wrT, RC)
    rT_pad = np.zeros((R_PAD, B), dtype=bf)
    rT_pad[:R, :] = np.ascontiguousarray(r.T).astype(bf)

    xT = np.ascontiguousarray(x.T).astype(f8)     # (E, B)
    yT = np.ascontiguousarray(y.T).astype(f8)
    weT = (np.ascontiguousarray(W_e.T) * S_WE).astype(f8)  # (E, D)

    in_maps = []
    for c in range(NCORES):
        lo, hi = c * E_SH, (c + 1) * E_SH
        xT_sh = np.zeros((E_PAD, B), dtype=f8)
        xT_sh[:E_SH] = xT[lo:hi]
        yT_sh = np.zeros((E_PAD, B), dtype=f8)
        yT_sh[:E_SH] = yT[lo:hi]
        weT_sh = np.zeros((E_PAD, D), dtype=f8)
        weT_sh[:E_SH] = weT[lo:hi]
        xp = _pmajor(xT_sh, KC)     # (128, KC, B)
        yp = _pmajor(yT_sh, KC)
        m = {
            "weT": _pmajor(weT_sh, KC),
            "rT": _pmajor(np.ascontiguousarray(rT_pad[:, _core_rows(c)]),
                          RC),
            "wrT": wrT_p,
            "fab": fab_p,
            "fr": fr_p,
        }
        for ci, (col0, ncols) in enumerate(CHUNKS):
            m[f"x{ci}"] = np.ascontiguousarray(xp[:, :, col0:col0 + ncols])
            m[f"y{ci}"] = np.ascontiguousarray(yp[:, :, col0:col0 + ncols])
        in_maps.append(m)

    res = run_bass_kernel_spmd(nc, in_maps, core_ids=list(range(NCORES)))
    out = np.empty((B, 1), dtype=np.float32)
    for c in range(NCORES):
        out[_core_rows(c)] = res.results[c]["out"]
    return out
